# revision 3
# baseline (speedup 1.0000x reference)
"""Multi-head causal self-attention for TRN2, 8 NeuronCores.

Sharding: core i handles (batch b = i//2, head-group g = i%2); each head-group
is 8 of the 16 heads.  Per core everything is computed in "transposed" space so
no on-device transposes are needed:

  phase 1:  Q^T, K^T [512, T] = W_{q,k}^T @ x^T   (lhsT = W rows, rhs = x^T)
            V [T, 512] = x @ W_v                  (lhsT = x^T chunk, rhs = W_v)
            V staged as [V_h | ones] per head for the softmax row-sum trick
  phase 2:  per (head, tq-block 512, tk-chunk 128):
            S^T = K^T_chunk.T @ Q^T   (K=64 matmul, packed head pairs on
                                       partitions 0:64 / 64:128)
            P^T = exp(S^T/8) via ACT (PSUM -> SBUF fp32r)
            causal mask multiply on diagonal chunks (DVE)
            O^T(+sums) accumulate:  [V_h | 1].T @ P^T  in PSUM [65, 512]
            normalize: recip(sums) + K=1 broadcast matmul + DVE multiply
  phase 3:  partial out [T, D] = Y^T.T @ W_proj_rows ; host sums the two
            head-group partials per batch and adds b_proj.

All matmuls run in float32r (4x faster than fp32 on the PE, ~1.5e-4 rel err).
"""

import numpy as np
from contextlib import ExitStack

import concourse.bass as bass
import concourse.mybir as mybir
import concourse.tile as tile
from concourse import bacc
from concourse.bass_utils import run_bass_kernel_spmd

B, T, D, H = 4, 2048, 1024, 16
DK = 64            # head dim
HL = 8             # heads per core
DL = HL * DK       # 512 local head dims per core
N_CORES = 8

F32 = mybir.dt.float32
F32R = mybir.dt.float32r
EXP = mybir.ActivationFunctionType.Exp
IDENT = mybir.ActivationFunctionType.Identity

TQ = 512           # tq block size
TKC = 128          # tk chunk size
NQB = T // TQ      # 4
NKC = T // TKC     # 16
NDCH = D // 128    # 8 contraction chunks over D

_CACHE = {}


def _build(causal: bool):
    nc = bacc.Bacc("TRN2", target_bir_lowering=False, debug=False,
                   num_devices=N_CORES)
    xT_d = nc.dram_tensor("xT", [D, T], F32, kind="ExternalInput").ap()
    wqk_d = nc.dram_tensor("wqk", [D, 2 * DL], F32, kind="ExternalInput").ap()
    wv_d = nc.dram_tensor("wv", [D, DL], F32, kind="ExternalInput").ap()
    bqk_d = nc.dram_tensor("bqk", [2 * DL // 128, 128, 1], F32,
                           kind="ExternalInput").ap()
    bv_d = nc.dram_tensor("bv", [1, DL], F32, kind="ExternalInput").ap()
    wp_d = nc.dram_tensor("wproj", [DL, D], F32, kind="ExternalInput").ap()
    masks_d = nc.dram_tensor("masks", [TKC, 4 * TQ], F32, kind="ExternalInput").ap()
    out_d = nc.dram_tensor("out", [T, D], F32, kind="ExternalOutput").ap()

    with tile.TileContext(nc) as tc, ExitStack() as top:
        persist = top.enter_context(tc.tile_pool(name="persist", bufs=1))

        # persistent fp32r tensors (head pairs packed on partitions)
        qT = [persist.tile([128, T], F32R, tag=f"qT{i}", name=f"qT{i}")
              for i in range(4)]
        kT = [persist.tile([128, T], F32R, tag=f"kT{i}", name=f"kT{i}")
              for i in range(4)]
        vs = [persist.tile([128, HL * 65], F32R, tag=f"vs{t}", name=f"vs{t}")
              for t in range(NKC)]
        ones_r = persist.tile([1, 128], F32R, tag="ones_r", name="ones_r")
        maskr = None
        if causal:
            maskr = persist.tile([TKC, 4 * TQ], F32R, tag="maskr", name="maskr")

        # small constants / bias tiles
        ones_f = persist.tile([1, 128], F32, tag="ones_f", name="ones_f")
        nc.vector.memset(ones_f[:], 1.0)
        nc.vector.tensor_copy(ones_r[:], ones_f[:])
        ones8 = persist.tile([128, 1], F32, tag="ones8", name="ones8")
        nc.vector.memset(ones8[:], 1.0)

        bqk_sb = [persist.tile([128, 1], F32, tag=f"bqk{m}", name=f"bqk{m}")
                  for m in range(8)]
        for m in range(8):
            nc.sync.dma_start(bqk_sb[m][:], bqk_d[m])
        bv_f = persist.tile([1, DL], F32, tag="bv_f", name="bv_f")
        nc.sync.dma_start(bv_f[:], bv_d)
        bv_r = persist.tile([1, DL], F32R, tag="bv_r", name="bv_r")
        nc.vector.tensor_copy(bv_r[:], bv_f[:])

        if causal:
            with tc.tile_pool(name="mstage_pool", bufs=1) as mstage_pool:
                mstage = mstage_pool.tile([TKC, 4 * TQ], F32, tag="mstage",
                                          name="mstage")
                nc.sync.dma_start(mstage[:], masks_d)
                nc.vector.tensor_copy(maskr[:], mstage[:])

        # ---------------- phase 1: QKV projections ----------------
        with ExitStack() as ph1:
            wstage = ph1.enter_context(tc.tile_pool(name="wstage", bufs=1))
            wpool = ph1.enter_context(tc.tile_pool(name="wpool", bufs=1))
            xstage = ph1.enter_context(tc.tile_pool(name="xstage", bufs=2))
            xrpool = ph1.enter_context(tc.tile_pool(name="xrpool", bufs=1))
            ps1 = ph1.enter_context(tc.tile_pool(name="ps1", bufs=3, space="PSUM"))

            wqk_r, wv_r = [], []
            for d in range(NDCH):
                st = wstage.tile([128, 2 * DL], F32, tag="wqks", name=f"wqks{d}")
                nc.sync.dma_start(st[:], wqk_d[d * 128:(d + 1) * 128, :])
                wr = wpool.tile([128, 2 * DL], F32R, tag=f"wqk{d}", name=f"wqk{d}")
                nc.vector.tensor_copy(wr[:], st[:])
                wqk_r.append(wr)

                stv = wstage.tile([128, DL], F32, tag="wvs", name=f"wvs{d}")
                nc.sync.dma_start(stv[:], wv_d[d * 128:(d + 1) * 128, :])
                wvr = wpool.tile([128, DL], F32R, tag=f"wv{d}", name=f"wv{d}")
                nc.vector.tensor_copy(wvr[:], stv[:])
                wv_r.append(wvr)

            for j in range(NQB):
                xr = []
                for d in range(NDCH):
                    st = xstage.tile([128, TQ], F32, tag="xs", name=f"xs{j}_{d}")
                    nc.sync.dma_start(
                        st[:], xT_d[d * 128:(d + 1) * 128, j * TQ:(j + 1) * TQ])
                    xrt = xrpool.tile([128, TQ], F32R, tag=f"xr{d}",
                                      name=f"xr{j}_{d}")
                    nc.vector.tensor_copy(xrt[:], st[:])
                    xr.append(xrt)

                # Q^T / K^T m-chunks (m 0..3 -> qT, 4..7 -> kT)
                for m in range(8):
                    ps = ps1.tile([128, TQ], F32, tag="psqk", name=f"psqk{j}_{m}")
                    for d in range(NDCH):
                        nc.tensor.matmul(
                            ps[:], wqk_r[d][:, m * 128:(m + 1) * 128], xr[d][:],
                            start=(d == 0), stop=(d == NDCH - 1))
                    dst = qT[m] if m < 4 else kT[m - 4]
                    nc.scalar.activation(
                        dst[:, j * TQ:(j + 1) * TQ], ps[:], IDENT,
                        bias=bqk_sb[m][:], scale=1.0)

                # V t-chunks for this column block
                for tt in range(4 * j, 4 * j + 4):
                    c = tt % 4
                    ps = ps1.tile([128, DL], F32, tag="psv", name=f"psv{tt}")
                    for d in range(NDCH):
                        nc.tensor.matmul(
                            ps[:], xr[d][:, c * 128:(c + 1) * 128], wv_r[d][:],
                            start=(d == 0), stop=False)
                    nc.tensor.matmul(ps[:], ones_r[:, 0:128], bv_r[:],
                                     start=False, stop=True)
                    for h in range(HL):
                        nc.scalar.copy(vs[tt][:, h * 65:h * 65 + 64],
                                       ps[:, h * 64:(h + 1) * 64])
                        nc.scalar.copy(vs[tt][:, h * 65 + 64:h * 65 + 65],
                                       ones8[:])

        # ---------------- phase 2: attention ----------------
        ypool = top.enter_context(tc.tile_pool(name="ypool", bufs=1))
        yT = [ypool.tile([128, T], F32R, tag=f"yT{i}", name=f"yT{i}")
              for i in range(4)]
        with ExitStack() as ph2:
            ps_s = ph2.enter_context(tc.tile_pool(name="ps_s", bufs=3, space="PSUM"))
            ps_o = ph2.enter_context(tc.tile_pool(name="ps_o", bufs=2, space="PSUM"))
            ps_b = ph2.enter_context(tc.tile_pool(name="ps_b", bufs=2, space="PSUM"))
            ppool = ph2.enter_context(tc.tile_pool(name="ppool", bufs=4))
            npool = ph2.enter_context(tc.tile_pool(name="npool", bufs=3))

            for j in range(NQB):
                cs = list(range(4 * (j + 1))) if causal else list(range(NKC))
                for h in range(HL):
                    i, hp = h // 2, h % 2
                    q_ap = qT[i][hp * 64:(hp + 1) * 64, j * TQ:(j + 1) * TQ]
                    po = ps_o.tile([65, TQ], F32, tag="po", name=f"po{j}_{h}")

                    pending = None  # software-pipeline: PV(c) after QK(c+1)
                    for ci, c in enumerate(cs):
                        k_ap = kT[i][hp * 64:(hp + 1) * 64,
                                     c * TKC:(c + 1) * TKC]
                        ss = ps_s.tile([TKC, TQ], F32, tag="ss",
                                       name=f"ss{j}_{h}_{c}")
                        nc.tensor.matmul(ss[:], k_ap, q_ap, start=True, stop=True)
                        pt = ppool.tile([TKC, TQ], F32R, tag="pt",
                                        name=f"pt{j}_{h}_{c}")
                        nc.scalar.activation(pt[:], ss[:], EXP, scale=0.125)
                        if causal and c >= 4 * j:
                            s = c - 4 * j
                            nc.vector.tensor_mul(
                                pt[:], pt[:], maskr[:, s * TQ:(s + 1) * TQ])
                        if pending is not None:
                            nc.tensor.matmul(po[:], *pending,
                                             start=(ci == 1), stop=False)
                        pending = (vs[c][:, h * 65:h * 65 + 65], pt[:])
                    nc.tensor.matmul(po[:], *pending,
                                     start=(len(cs) == 1), stop=True)

                    # normalize
                    recip = npool.tile([1, TQ], F32, tag="recip",
                                       name=f"rc{j}_{h}")
                    nc.vector.reciprocal(recip[:], po[64:65, :])
                    recip_r = npool.tile([1, TQ], F32R, tag="recip_r",
                                         name=f"rr{j}_{h}")
                    nc.vector.tensor_copy(recip_r[:], recip[:])
                    o_sb = npool.tile([64, TQ], F32, tag="o_sb",
                                      name=f"ob{j}_{h}")
                    nc.vector.tensor_copy(o_sb[:], po[0:64, :])
                    pb = ps_b.tile([64, TQ], F32, tag="pb", name=f"pb{j}_{h}")
                    nc.tensor.matmul(pb[:], ones_r[:, 0:64], recip_r[:],
                                     start=True, stop=True)
                    nc.vector.tensor_mul(
                        yT[i][hp * 64:(hp + 1) * 64, j * TQ:(j + 1) * TQ],
                        o_sb[:], pb[:])

        # ---------------- phase 3: output projection ----------------
        with ExitStack() as ph3:
            wstage3 = ph3.enter_context(tc.tile_pool(name="wstage3", bufs=2))
            wpool3 = ph3.enter_context(tc.tile_pool(name="wpool3", bufs=1))
            opool = ph3.enter_context(tc.tile_pool(name="opool", bufs=3))
            ps3 = ph3.enter_context(tc.tile_pool(name="ps3", bufs=4, space="PSUM"))

            wp_r = []
            for k in range(4):
                st = wstage3.tile([128, D], F32, tag="wps", name=f"wps{k}")
                nc.sync.dma_start(st[:], wp_d[k * 128:(k + 1) * 128, :])
                wr = wpool3.tile([128, D], F32R, tag=f"wp{k}", name=f"wp{k}")
                nc.vector.tensor_copy(wr[:], st[:])
                wp_r.append(wr)

            for t in range(T // 128):
                ot = opool.tile([128, D], F32, tag="ot", name=f"ot{t}")
                for nb in range(2):
                    ps = ps3.tile([128, 512], F32, tag="ps3",
                                  name=f"ps3_{t}_{nb}")
                    for k in range(4):
                        nc.tensor.matmul(
                            ps[:],
                            yT[k][:, t * 128:(t + 1) * 128],
                            wp_r[k][:, nb * 512:(nb + 1) * 512],
                            start=(k == 0), stop=(k == 3))
                    nc.scalar.copy(ot[:, nb * 512:(nb + 1) * 512], ps[:])
                nc.sync.dma_start(out_d[t * 128:(t + 1) * 128, :], ot[:])

    nc.compile()
    return nc


def _get_nc(causal: bool):
    if causal not in _CACHE:
        _CACHE[causal] = _build(causal)
    return _CACHE[causal]


def _host_masks() -> np.ndarray:
    i = np.arange(TKC)[:, None]
    jj = np.arange(TQ)[None, :]
    blocks = [(jj >= i + s * TKC).astype(np.float32) for s in range(4)]
    return np.ascontiguousarray(np.concatenate(blocks, axis=1))


def kernel(x, mask, W_qkv, b_qkv, W_proj, b_proj):
    x = np.asarray(x, dtype=np.float32)
    mask2d = np.asarray(mask, dtype=np.int32).reshape(T, T)
    W_qkv = np.asarray(W_qkv, dtype=np.float32)
    b_qkv = np.asarray(b_qkv, dtype=np.float32)
    W_proj = np.asarray(W_proj, dtype=np.float32)
    b_proj = np.asarray(b_proj, dtype=np.float32)

    if np.array_equal(mask2d, np.tril(np.ones((T, T), dtype=np.int32))):
        causal = True
    elif np.all(mask2d == 1):
        causal = False
    else:
        raise NotImplementedError("only causal (tril) or all-ones masks")

    nc = _get_nc(causal)
    masks_np = _host_masks()

    in_maps = []
    for core in range(N_CORES):
        b, g = core // 2, core % 2
        qc = slice(g * DL, (g + 1) * DL)
        kc = slice(D + g * DL, D + (g + 1) * DL)
        vc = slice(2 * D + g * DL, 2 * D + (g + 1) * DL)
        in_maps.append({
            "xT": np.ascontiguousarray(x[b].T),
            "wqk": np.ascontiguousarray(
                np.concatenate([W_qkv[:, qc], W_qkv[:, kc]], axis=1)),
            "wv": np.ascontiguousarray(W_qkv[:, vc]),
            "bqk": np.ascontiguousarray(
                np.concatenate([b_qkv[qc], b_qkv[kc]]).reshape(8, 128, 1)),
            "bv": np.ascontiguousarray(b_qkv[vc].reshape(1, DL)),
            "wproj": np.ascontiguousarray(W_proj[g * DL:(g + 1) * DL, :]),
            "masks": masks_np,
        })

    res = run_bass_kernel_spmd(nc, in_maps, core_ids=list(range(N_CORES)))
    out = np.empty((B, T, D), dtype=np.float32)
    for b in range(B):
        out[b] = (res.results[2 * b]["out"] + res.results[2 * b + 1]["out"]
                  + b_proj[None, :])
    return out


# revision 5
# speedup vs baseline: 1.0453x; 1.0453x over previous
"""Multi-head causal self-attention for TRN2, 8 NeuronCores.

Sharding: core i handles (batch b = i//2, head-group g = i%2); each head-group
is 8 of the 16 heads.  Per core everything is computed in "transposed" space so
no on-device transposes are needed:

  phase 1:  Q^T, K^T [512, T] = W_{q,k}^T @ x^T   (lhsT = W rows, rhs = x^T)
            V [T, 512] = x @ W_v                  (lhsT = x^T chunk, rhs = W_v)
            Q^T stored per-head zero-padded to 128 partitions so attention
            matmuls are full 128x128 shapes (keeps the PE HAM clock-gate warm);
            V staged bf16 as [V_h | ones] per head for the softmax row-sum trick
  phase 2:  per (head-pair, tq-block 512, tk-chunk 128):
            S^T(2 heads) = kT_chunk.T @ qTp  into one [128,1024] PSUM tile
            P^T = exp(S^T/8) via one ACT op -> bf16
            causal mask multiply on diagonal chunks (DVE, bf16 4x)
            O^T(+sums) accumulate per head:  [V_h|1|..].T @ P^T  in PSUM [128,512]
            normalize: approx-reciprocal(sums) + K=1 broadcast matmul + DVE mul
  phase 3:  partial out [T, D] = Y^T.T @ W_proj_rows ; host sums the two
            head-group partials per batch and adds b_proj.

Matmuls run in float32r (4x faster than fp32, ~1.5e-4 rel err); P/V in bf16.
"""

import numpy as np
from contextlib import ExitStack

import concourse.bass as bass
import concourse.mybir as mybir
import concourse.tile as tile
from concourse import bacc
from concourse.bass_utils import run_bass_kernel_spmd

B, T, D, H = 4, 2048, 1024, 16
DK = 64            # head dim
HL = 8             # heads per core
DL = HL * DK       # 512 local head dims per core
N_CORES = 8

F32 = mybir.dt.float32
F32R = mybir.dt.float32r
BF16 = mybir.dt.bfloat16
EXP = mybir.ActivationFunctionType.Exp
IDENT = mybir.ActivationFunctionType.Identity

TQ = 512           # tq block size
TKC = 128          # tk chunk size
NQB = T // TQ      # 4
NKC = T // TKC     # 16
NDCH = D // 128    # 8 contraction chunks over D
VSW = HL * 65 + 64  # staged-V width: 8*[V_h|1] + ones tail pad for M=128 lhsT

_CACHE = {}


def _build(causal: bool):
    nc = bacc.Bacc("TRN2", target_bir_lowering=False, debug=False,
                   num_devices=N_CORES)
    xT_d = nc.dram_tensor("xT", [D, T], F32, kind="ExternalInput").ap()
    wqk_d = nc.dram_tensor("wqk", [D, 2 * DL], F32, kind="ExternalInput").ap()
    wv_d = nc.dram_tensor("wv", [D, DL], F32, kind="ExternalInput").ap()
    bqk_d = nc.dram_tensor("bqk", [2 * DL // 128, 128, 1], F32,
                           kind="ExternalInput").ap()
    bv_d = nc.dram_tensor("bv", [1, DL], F32, kind="ExternalInput").ap()
    wp_d = nc.dram_tensor("wproj", [DL, D], F32, kind="ExternalInput").ap()
    masks_d = nc.dram_tensor("masks", [TKC, 4 * TQ], F32, kind="ExternalInput").ap()
    out_d = nc.dram_tensor("out", [T, D], F32, kind="ExternalOutput").ap()

    with tile.TileContext(nc) as tc, ExitStack() as top:
        persist = top.enter_context(tc.tile_pool(name="persist", bufs=1))

        # persistent tensors
        qTp = [persist.tile([128, T], F32R, tag=f"qTp{h}", name=f"qTp{h}")
               for h in range(HL)]      # per-head, zero-padded other half
        kT = [persist.tile([128, T], F32R, tag=f"kT{i}", name=f"kT{i}")
              for i in range(4)]        # head-pair packed
        vs = [persist.tile([128, VSW], BF16, tag=f"vs{t}", name=f"vs{t}")
              for t in range(NKC)]
        ones_r = persist.tile([1, 128], F32R, tag="ones_r", name="ones_r")
        maskb = None
        if causal:
            maskb = persist.tile([TKC, 4 * TQ], BF16, tag="maskb", name="maskb")

        ones_f = persist.tile([1, 128], F32, tag="ones_f", name="ones_f")
        nc.vector.memset(ones_f[:], 1.0)
        nc.vector.tensor_copy(ones_r[:], ones_f[:])
        ones8 = persist.tile([128, 64], F32, tag="ones8", name="ones8")
        nc.vector.memset(ones8[:], 1.0)

        bqk_sb = [persist.tile([128, 1], F32, tag=f"bqk{m}", name=f"bqk{m}")
                  for m in range(8)]
        for m in range(8):
            nc.sync.dma_start(bqk_sb[m][:], bqk_d[m])
        bv_f = persist.tile([1, DL], F32, tag="bv_f", name="bv_f")
        nc.sync.dma_start(bv_f[:], bv_d)
        bv_r = persist.tile([1, DL], F32R, tag="bv_r", name="bv_r")
        nc.vector.tensor_copy(bv_r[:], bv_f[:])

        # one-time fills (scoped staging)
        with tc.tile_pool(name="init_pool", bufs=1) as initp:
            if causal:
                mstage = initp.tile([TKC, 4 * TQ], F32, tag="mstage",
                                    name="mstage")
                nc.sync.dma_start(mstage[:], masks_d)
                nc.vector.tensor_copy(maskb[:], mstage[:])
            zeros = initp.tile([64, T], F32, tag="zeros", name="zeros")
            nc.vector.memset(zeros[:], 0.0)
            for h in range(HL):
                hp = h % 2
                pad = slice(64, 128) if hp == 0 else slice(0, 64)
                nc.vector.tensor_copy(qTp[h][pad, :], zeros[:])
            # staged-V ones columns + tail pad
            for t in range(NKC):
                for h in range(HL):
                    nc.vector.tensor_copy(
                        vs[t][:, h * 65 + 64:h * 65 + 65], ones8[:, 0:1])
                nc.vector.tensor_copy(vs[t][:, HL * 65:VSW], ones8[:])

        # ---------------- phase 1: QKV projections ----------------
        with ExitStack() as ph1:
            wstage = ph1.enter_context(tc.tile_pool(name="wstage", bufs=1))
            wpool = ph1.enter_context(tc.tile_pool(name="wpool", bufs=1))
            xstage = ph1.enter_context(tc.tile_pool(name="xstage", bufs=2))
            xrpool = ph1.enter_context(tc.tile_pool(name="xrpool", bufs=1))
            ps1 = ph1.enter_context(tc.tile_pool(name="ps1", bufs=3, space="PSUM"))

            wqk_r, wv_r = [], []
            for d in range(NDCH):
                st = wstage.tile([128, 2 * DL], F32, tag="wqks", name=f"wqks{d}")
                nc.sync.dma_start(st[:], wqk_d[d * 128:(d + 1) * 128, :])
                wr = wpool.tile([128, 2 * DL], F32R, tag=f"wqk{d}", name=f"wqk{d}")
                nc.vector.tensor_copy(wr[:], st[:])
                wqk_r.append(wr)

                stv = wstage.tile([128, DL], F32, tag="wvs", name=f"wvs{d}")
                nc.sync.dma_start(stv[:], wv_d[d * 128:(d + 1) * 128, :])
                wvr = wpool.tile([128, DL], F32R, tag=f"wv{d}", name=f"wv{d}")
                nc.vector.tensor_copy(wvr[:], stv[:])
                wv_r.append(wvr)

            for j in range(NQB):
                xr = []
                for d in range(NDCH):
                    st = xstage.tile([128, TQ], F32, tag="xs", name=f"xs{j}_{d}")
                    nc.sync.dma_start(
                        st[:], xT_d[d * 128:(d + 1) * 128, j * TQ:(j + 1) * TQ])
                    xrt = xrpool.tile([128, TQ], F32R, tag=f"xr{d}",
                                      name=f"xr{j}_{d}")
                    nc.vector.tensor_copy(xrt[:], st[:])
                    xr.append(xrt)

                # Q^T / K^T m-chunks (m 0..3 -> qTp pairs, 4..7 -> kT)
                for m in range(8):
                    ps = ps1.tile([128, TQ], F32, tag="psqk", name=f"psqk{j}_{m}")
                    for d in range(NDCH):
                        nc.tensor.matmul(
                            ps[:], wqk_r[d][:, m * 128:(m + 1) * 128], xr[d][:],
                            start=(d == 0), stop=(d == NDCH - 1))
                    jsl = slice(j * TQ, (j + 1) * TQ)
                    if m < 4:
                        # rows 0:64 -> head 2m (partition 0:64 of qTp[2m]);
                        # rows 64:128 -> head 2m+1 (partition 64:128 of qTp[2m+1])
                        nc.scalar.activation(
                            qTp[2 * m][0:64, jsl], ps[0:64, :], IDENT,
                            bias=bqk_sb[m][0:64], scale=1.0)
                        nc.scalar.activation(
                            qTp[2 * m + 1][64:128, jsl], ps[64:128, :], IDENT,
                            bias=bqk_sb[m][64:128], scale=1.0)
                    else:
                        nc.scalar.activation(
                            kT[m - 4][:, jsl], ps[:], IDENT,
                            bias=bqk_sb[m][:], scale=1.0)

                # V t-chunks for this column block
                for tt in range(4 * j, 4 * j + 4):
                    c = tt % 4
                    ps = ps1.tile([128, DL], F32, tag="psv", name=f"psv{tt}")
                    for d in range(NDCH):
                        nc.tensor.matmul(
                            ps[:], xr[d][:, c * 128:(c + 1) * 128], wv_r[d][:],
                            start=(d == 0), stop=False)
                    nc.tensor.matmul(ps[:], ones_r[:, 0:128], bv_r[:],
                                     start=False, stop=True)
                    for h in range(HL):
                        nc.scalar.copy(vs[tt][:, h * 65:h * 65 + 64],
                                       ps[:, h * 64:(h + 1) * 64])

        # ---------------- phase 2: attention ----------------
        ypool = top.enter_context(tc.tile_pool(name="ypool", bufs=1))
        yT = [ypool.tile([128, T], F32R, tag=f"yT{i}", name=f"yT{i}")
              for i in range(4)]
        with ExitStack() as ph2:
            ps_s = ph2.enter_context(tc.tile_pool(name="ps_s", bufs=2, space="PSUM"))
            ps_o = ph2.enter_context(tc.tile_pool(name="ps_o", bufs=4, space="PSUM"))
            ppool = ph2.enter_context(tc.tile_pool(name="ppool", bufs=3))
            npool = ph2.enter_context(tc.tile_pool(name="npool", bufs=2))

            for j in range(NQB):
                cs = list(range(4 * (j + 1))) if causal else list(range(NKC))
                for i in range(4):          # head pair (2i, 2i+1)
                    hA, hB = 2 * i, 2 * i + 1
                    poA = ps_o.tile([128, TQ], F32, tag="po", name=f"poA{j}_{i}")
                    poB = ps_o.tile([128, TQ], F32, tag="po", name=f"poB{j}_{i}")
                    jsl = slice(j * TQ, (j + 1) * TQ)

                    pend = None   # pipeline: PV(c) emitted after QK(c+1)
                    for ci, c in enumerate(cs):
                        csl = slice(c * TKC, (c + 1) * TKC)
                        ss = ps_s.tile([TKC, 2 * TQ], F32, tag="ss",
                                       name=f"ss{j}_{i}_{c}")
                        nc.tensor.matmul(ss[:, 0:TQ], kT[i][:, csl],
                                         qTp[hA][:, jsl], start=True, stop=True)
                        nc.tensor.matmul(ss[:, TQ:2 * TQ], kT[i][:, csl],
                                         qTp[hB][:, jsl], start=True, stop=True)
                        pt = ppool.tile([TKC, 2 * TQ], BF16, tag="pt",
                                        name=f"pt{j}_{i}_{c}")
                        nc.scalar.activation(pt[:], ss[:], EXP, scale=0.125)
                        if causal and c >= 4 * j:
                            s = c - 4 * j
                            msl = slice(s * TQ, (s + 1) * TQ)
                            nc.vector.tensor_mul(pt[:, 0:TQ], pt[:, 0:TQ],
                                                 maskb[:, msl])
                            nc.vector.tensor_mul(pt[:, TQ:2 * TQ],
                                                 pt[:, TQ:2 * TQ], maskb[:, msl])
                        if pend is not None:
                            pc, ppt = pend
                            st = (ci == 1)
                            nc.tensor.matmul(
                                poA[:], vs[pc][:, hA * 65:hA * 65 + 128],
                                ppt[:, 0:TQ], start=st, stop=False)
                            nc.tensor.matmul(
                                poB[:], vs[pc][:, hB * 65:hB * 65 + 128],
                                ppt[:, TQ:2 * TQ], start=st, stop=False)
                        pend = (c, pt)
                    pc, ppt = pend
                    one = (len(cs) == 1)
                    nc.tensor.matmul(poA[:], vs[pc][:, hA * 65:hA * 65 + 128],
                                     ppt[:, 0:TQ], start=one, stop=True)
                    nc.tensor.matmul(poB[:], vs[pc][:, hB * 65:hB * 65 + 128],
                                     ppt[:, TQ:2 * TQ], start=one, stop=True)

                    # normalize both heads
                    for h, po in ((hA, poA), (hB, poB)):
                        hp = h % 2
                        recip = npool.tile([1, TQ], F32, tag="recip",
                                           name=f"rc{j}_{h}")
                        nc.vector.reciprocal(recip[:], po[64:65, :])
                        recip_r = npool.tile([1, TQ], F32R, tag="recip_r",
                                             name=f"rr{j}_{h}")
                        nc.vector.tensor_copy(recip_r[:], recip[:])
                        o_sb = npool.tile([64, TQ], F32, tag="o_sb",
                                          name=f"ob{j}_{h}")
                        nc.vector.tensor_copy(o_sb[:], po[0:64, :])
                        pb = ps_o.tile([64, TQ], F32, tag="po",
                                       name=f"pb{j}_{h}")
                        nc.tensor.matmul(pb[:], ones_r[:, 0:64], recip_r[:],
                                         start=True, stop=True)
                        nc.vector.tensor_mul(
                            yT[i][hp * 64:(hp + 1) * 64, jsl], o_sb[:], pb[:])

        # ---------------- phase 3: output projection ----------------
        with ExitStack() as ph3:
            wstage3 = ph3.enter_context(tc.tile_pool(name="wstage3", bufs=2))
            wpool3 = ph3.enter_context(tc.tile_pool(name="wpool3", bufs=1))
            opool = ph3.enter_context(tc.tile_pool(name="opool", bufs=3))
            ps3 = ph3.enter_context(tc.tile_pool(name="ps3", bufs=4, space="PSUM"))

            wp_r = []
            for k in range(4):
                st = wstage3.tile([128, D], F32, tag="wps", name=f"wps{k}")
                nc.sync.dma_start(st[:], wp_d[k * 128:(k + 1) * 128, :])
                wr = wpool3.tile([128, D], F32R, tag=f"wp{k}", name=f"wp{k}")
                nc.vector.tensor_copy(wr[:], st[:])
                wp_r.append(wr)

            for t in range(T // 128):
                ot = opool.tile([128, D], F32, tag="ot", name=f"ot{t}")
                for nb in range(2):
                    ps = ps3.tile([128, 512], F32, tag="ps3",
                                  name=f"ps3_{t}_{nb}")
                    for k in range(4):
                        nc.tensor.matmul(
                            ps[:],
                            yT[k][:, t * 128:(t + 1) * 128],
                            wp_r[k][:, nb * 512:(nb + 1) * 512],
                            start=(k == 0), stop=(k == 3))
                    nc.scalar.copy(ot[:, nb * 512:(nb + 1) * 512], ps[:])
                nc.sync.dma_start(out_d[t * 128:(t + 1) * 128, :], ot[:])

    nc.compile()
    return nc


def _get_nc(causal: bool):
    if causal not in _CACHE:
        _CACHE[causal] = _build(causal)
    return _CACHE[causal]


def _host_masks() -> np.ndarray:
    i = np.arange(TKC)[:, None]
    jj = np.arange(TQ)[None, :]
    blocks = [(jj >= i + s * TKC).astype(np.float32) for s in range(4)]
    return np.ascontiguousarray(np.concatenate(blocks, axis=1))


def _make_in_maps(x, W_qkv, b_qkv, W_proj):
    masks_np = _host_masks()
    in_maps = []
    for core in range(N_CORES):
        b, g = core // 2, core % 2
        qc = slice(g * DL, (g + 1) * DL)
        kc = slice(D + g * DL, D + (g + 1) * DL)
        vc = slice(2 * D + g * DL, 2 * D + (g + 1) * DL)
        in_maps.append({
            "xT": np.ascontiguousarray(x[b].T),
            "wqk": np.ascontiguousarray(
                np.concatenate([W_qkv[:, qc], W_qkv[:, kc]], axis=1)),
            "wv": np.ascontiguousarray(W_qkv[:, vc]),
            "bqk": np.ascontiguousarray(
                np.concatenate([b_qkv[qc], b_qkv[kc]]).reshape(8, 128, 1)),
            "bv": np.ascontiguousarray(b_qkv[vc].reshape(1, DL)),
            "wproj": np.ascontiguousarray(W_proj[g * DL:(g + 1) * DL, :]),
            "masks": masks_np,
        })
    return in_maps


def kernel(x, mask, W_qkv, b_qkv, W_proj, b_proj):
    x = np.asarray(x, dtype=np.float32)
    mask2d = np.asarray(mask, dtype=np.int32).reshape(T, T)
    W_qkv = np.asarray(W_qkv, dtype=np.float32)
    b_qkv = np.asarray(b_qkv, dtype=np.float32)
    W_proj = np.asarray(W_proj, dtype=np.float32)
    b_proj = np.asarray(b_proj, dtype=np.float32)

    if np.array_equal(mask2d, np.tril(np.ones((T, T), dtype=np.int32))):
        causal = True
    elif np.all(mask2d == 1):
        causal = False
    else:
        raise NotImplementedError("only causal (tril) or all-ones masks")

    nc = _get_nc(causal)
    in_maps = _make_in_maps(x, W_qkv, b_qkv, W_proj)
    res = run_bass_kernel_spmd(nc, in_maps, core_ids=list(range(N_CORES)))
    out = np.empty((B, T, D), dtype=np.float32)
    for b in range(B):
        out[b] = (res.results[2 * b]["out"] + res.results[2 * b + 1]["out"]
                  + b_proj[None, :])
    return out


# revision 8
# speedup vs baseline: 1.5193x; 1.4535x over previous
"""Multi-head causal self-attention for TRN2, 8 NeuronCores.

Sharding: core i handles (batch b = i//2, head-group g = i%2); each head-group
is 8 of the 16 heads.  Per core everything is computed in "transposed" space so
no on-device transposes are needed:

  phase 1:  Q^T, K^T [512, T] = W_{q,k}^T @ x^T   (lhsT = W rows, rhs = x^T)
            V [T, 512] = x @ W_v                  (lhsT = x^T chunk, rhs = W_v)
            Q^T stored per-head zero-padded to 128 partitions so attention
            matmuls are full 128x128 shapes (keeps the PE HAM clock-gate warm);
            V staged bf16 as [V_h | ones] per head for the softmax row-sum trick
  phase 2:  per (head-pair, tq-block 512, tk-chunk 128):
            S^T(2 heads) = kT_chunk.T @ qTp  into one [128,1024] PSUM tile
            P^T = exp(S^T/8) via one ACT op -> bf16
            causal mask multiply on diagonal chunks (DVE, bf16 4x)
            O^T(+sums) accumulate per head:  [V_h|1|..].T @ P^T  in PSUM [128,512]
            normalize: approx-reciprocal(sums) + K=1 broadcast matmul + DVE mul
  phase 3:  partial out [T, D] = Y^T.T @ W_proj_rows ; host sums the two
            head-group partials per batch and adds b_proj.

Matmuls run in float32r (4x faster than fp32, ~1.5e-4 rel err); P/V in bf16.
"""

import numpy as np
from contextlib import ExitStack

import concourse.bass as bass
import concourse.mybir as mybir
import concourse.tile as tile
from concourse import bacc
from concourse.bass_utils import run_bass_kernel_spmd

B, T, D, H = 4, 2048, 1024, 16
DK = 64            # head dim
HL = 8             # heads per core
DL = HL * DK       # 512 local head dims per core
N_CORES = 8

F32 = mybir.dt.float32
F32R = mybir.dt.float32r
BF16 = mybir.dt.bfloat16
EXP = mybir.ActivationFunctionType.Exp
IDENT = mybir.ActivationFunctionType.Identity

TQ = 512           # tq block size
TKC = 128          # tk chunk size
NQB = T // TQ      # 4
NKC = T // TKC     # 16
NDCH = D // 128    # 8 contraction chunks over D
VSW = HL * 65 + 64  # staged-V width: 8*[V_h|1] + ones tail pad for M=128 lhsT

_CACHE = {}


def _build(causal: bool):
    nc = bacc.Bacc("TRN2", target_bir_lowering=False, debug=False,
                   num_devices=N_CORES)
    xT_d = nc.dram_tensor("xT", [D, T], F32, kind="ExternalInput").ap()
    wqk_d = nc.dram_tensor("wqk", [D, 2 * DL], F32, kind="ExternalInput").ap()
    wv_d = nc.dram_tensor("wv", [D, DL], F32, kind="ExternalInput").ap()
    bqk_d = nc.dram_tensor("bqk", [2 * DL // 128, 128, 1], F32,
                           kind="ExternalInput").ap()
    bv_d = nc.dram_tensor("bv", [1, DL], F32, kind="ExternalInput").ap()
    wp_d = nc.dram_tensor("wproj", [DL, D], F32, kind="ExternalInput").ap()
    masks_d = nc.dram_tensor("masks", [TKC, 4 * TQ], F32, kind="ExternalInput").ap()
    out_d = nc.dram_tensor("out", [T, D], F32, kind="ExternalOutput").ap()

    with tile.TileContext(nc) as tc, ExitStack() as top:
        persist = top.enter_context(tc.tile_pool(name="persist", bufs=1))

        # persistent tensors
        qTp = [persist.tile([128, T], F32R, tag=f"qTp{h}", name=f"qTp{h}")
               for h in range(HL)]      # per-head, zero-padded other half
        kT = [persist.tile([128, T], F32R, tag=f"kT{i}", name=f"kT{i}")
              for i in range(4)]        # head-pair packed
        vs = [persist.tile([128, VSW], BF16, tag=f"vs{t}", name=f"vs{t}")
              for t in range(NKC)]
        ones_r = persist.tile([1, 128], F32R, tag="ones_r", name="ones_r")
        maskb = None
        if causal:
            maskb = persist.tile([TKC, 4 * TQ], BF16, tag="maskb", name="maskb")

        ones_f = persist.tile([1, 128], F32, tag="ones_f", name="ones_f")
        nc.vector.memset(ones_f[:], 1.0)
        nc.vector.tensor_copy(ones_r[:], ones_f[:])
        ones8 = persist.tile([128, 64], F32, tag="ones8", name="ones8")
        nc.vector.memset(ones8[:], 1.0)

        bqk_sb = [persist.tile([128, 1], F32, tag=f"bqk{m}", name=f"bqk{m}")
                  for m in range(8)]
        for m in range(8):
            nc.sync.dma_start(bqk_sb[m][:], bqk_d[m])
        bv_f = persist.tile([1, DL], F32, tag="bv_f", name="bv_f")
        nc.sync.dma_start(bv_f[:], bv_d)
        bv_r = persist.tile([1, DL], F32R, tag="bv_r", name="bv_r")
        nc.vector.tensor_copy(bv_r[:], bv_f[:])

        # one-time fills (scoped staging)
        with tc.tile_pool(name="init_pool", bufs=1) as initp:
            if causal:
                mstage = initp.tile([TKC, 4 * TQ], F32, tag="mstage",
                                    name="mstage")
                nc.sync.dma_start(mstage[:], masks_d)
                nc.vector.tensor_copy(maskb[:], mstage[:])
            zeros = initp.tile([64, T], F32, tag="zeros", name="zeros")
            nc.vector.memset(zeros[:], 0.0)
            for h in range(HL):
                hp = h % 2
                pad = slice(64, 128) if hp == 0 else slice(0, 64)
                nc.vector.tensor_copy(qTp[h][pad, :], zeros[:])
            # staged-V ones columns + tail pad
            for t in range(NKC):
                for h in range(HL):
                    nc.vector.tensor_copy(
                        vs[t][:, h * 65 + 64:h * 65 + 65], ones8[:, 0:1])
                nc.vector.tensor_copy(vs[t][:, HL * 65:VSW], ones8[:])

        # ---------------- phase 1: QKV projections ----------------
        with ExitStack() as ph1:
            wstage = ph1.enter_context(tc.tile_pool(name="wstage", bufs=1))
            wpool = ph1.enter_context(tc.tile_pool(name="wpool", bufs=1))
            xstage = ph1.enter_context(tc.tile_pool(name="xstage", bufs=2))
            xrpool = ph1.enter_context(tc.tile_pool(name="xrpool", bufs=1))
            ps1 = ph1.enter_context(tc.tile_pool(name="ps1", bufs=3, space="PSUM"))

            wqk_r, wv_r = [], []
            for d in range(NDCH):
                st = wstage.tile([128, 2 * DL], F32, tag="wqks", name=f"wqks{d}")
                nc.sync.dma_start(st[:], wqk_d[d * 128:(d + 1) * 128, :])
                wr = wpool.tile([128, 2 * DL], F32R, tag=f"wqk{d}", name=f"wqk{d}")
                nc.vector.tensor_copy(wr[:], st[:])
                wqk_r.append(wr)

                stv = wstage.tile([128, DL], F32, tag="wvs", name=f"wvs{d}")
                nc.sync.dma_start(stv[:], wv_d[d * 128:(d + 1) * 128, :])
                wvr = wpool.tile([128, DL], F32R, tag=f"wv{d}", name=f"wv{d}")
                nc.vector.tensor_copy(wvr[:], stv[:])
                wv_r.append(wvr)

            for j in range(NQB):
                xr = []
                for d in range(NDCH):
                    st = xstage.tile([128, TQ], F32, tag="xs", name=f"xs{j}_{d}")
                    nc.sync.dma_start(
                        st[:], xT_d[d * 128:(d + 1) * 128, j * TQ:(j + 1) * TQ])
                    xrt = xrpool.tile([128, TQ], F32R, tag=f"xr{d}",
                                      name=f"xr{j}_{d}")
                    nc.vector.tensor_copy(xrt[:], st[:])
                    xr.append(xrt)

                # Q^T / K^T m-chunks (m 0..3 -> qTp pairs, 4..7 -> kT)
                for m in range(8):
                    ps = ps1.tile([128, TQ], F32, tag="psqk", name=f"psqk{j}_{m}")
                    for d in range(NDCH):
                        nc.tensor.matmul(
                            ps[:], wqk_r[d][:, m * 128:(m + 1) * 128], xr[d][:],
                            start=(d == 0), stop=(d == NDCH - 1))
                    jsl = slice(j * TQ, (j + 1) * TQ)
                    if m < 4:
                        # rows 0:64 -> head 2m (partition 0:64 of qTp[2m]);
                        # rows 64:128 -> head 2m+1 (partition 64:128 of qTp[2m+1])
                        nc.scalar.activation(
                            qTp[2 * m][0:64, jsl], ps[0:64, :], IDENT,
                            bias=bqk_sb[m][0:64], scale=1.0)
                        nc.scalar.activation(
                            qTp[2 * m + 1][64:128, jsl], ps[64:128, :], IDENT,
                            bias=bqk_sb[m][64:128], scale=1.0)
                    else:
                        nc.scalar.activation(
                            kT[m - 4][:, jsl], ps[:], IDENT,
                            bias=bqk_sb[m][:], scale=1.0)

                # V t-chunks for this column block
                for tt in range(4 * j, 4 * j + 4):
                    c = tt % 4
                    ps = ps1.tile([128, DL], F32, tag="psv", name=f"psv{tt}")
                    for d in range(NDCH):
                        nc.tensor.matmul(
                            ps[:], xr[d][:, c * 128:(c + 1) * 128], wv_r[d][:],
                            start=(d == 0), stop=False)
                    nc.tensor.matmul(ps[:], ones_r[:, 0:128], bv_r[:],
                                     start=False, stop=True)
                    for h in range(HL):
                        nc.vector.tensor_copy(vs[tt][:, h * 65:h * 65 + 64],
                                              ps[:, h * 64:(h + 1) * 64])

        # ---------------- phase 2: attention ----------------
        ypool = top.enter_context(tc.tile_pool(name="ypool", bufs=1))
        yT = [ypool.tile([128, T], F32R, tag=f"yT{i}", name=f"yT{i}")
              for i in range(4)]
        with ExitStack() as ph2:
            ps_s = ph2.enter_context(tc.tile_pool(name="ps_s", bufs=2, space="PSUM"))
            ps_o = ph2.enter_context(tc.tile_pool(name="ps_o", bufs=3, space="PSUM"))
            ps_b = ph2.enter_context(tc.tile_pool(name="ps_b", bufs=1, space="PSUM"))
            ppool = ph2.enter_context(tc.tile_pool(name="ppool", bufs=3))
            npool = ph2.enter_context(tc.tile_pool(name="npool", bufs=2))

            for j in range(NQB):
                cs = list(range(4 * (j + 1))) if causal else list(range(NKC))
                for i in range(4):          # head pair (2i, 2i+1)
                    hA, hB = 2 * i, 2 * i + 1
                    poA = ps_o.tile([128, TQ], F32, tag="po", name=f"poA{j}_{i}")
                    poB = ps_o.tile([128, TQ], F32, tag="po", name=f"poB{j}_{i}")
                    jsl = slice(j * TQ, (j + 1) * TQ)

                    pend = None   # pipeline: PV(c) emitted after QK(c+1)
                    for ci, c in enumerate(cs):
                        csl = slice(c * TKC, (c + 1) * TKC)
                        ss = ps_s.tile([TKC, 2 * TQ], F32, tag="ss",
                                       name=f"ss{j}_{i}_{c}")
                        nc.tensor.matmul(ss[:, 0:TQ], kT[i][:, csl],
                                         qTp[hA][:, jsl], start=True, stop=True)
                        nc.tensor.matmul(ss[:, TQ:2 * TQ], kT[i][:, csl],
                                         qTp[hB][:, jsl], start=True, stop=True)
                        pt = ppool.tile([TKC, 2 * TQ], BF16, tag="pt",
                                        name=f"pt{j}_{i}_{c}")
                        nc.scalar.activation(pt[:], ss[:], EXP, scale=0.125)
                        if causal and c >= 4 * j:
                            s = c - 4 * j
                            msl = slice(s * TQ, (s + 1) * TQ)
                            nc.vector.tensor_mul(pt[:, 0:TQ], pt[:, 0:TQ],
                                                 maskb[:, msl])
                            nc.vector.tensor_mul(pt[:, TQ:2 * TQ],
                                                 pt[:, TQ:2 * TQ], maskb[:, msl])
                        if pend is not None:
                            pc, ppt = pend
                            st = (ci == 1)
                            nc.tensor.matmul(
                                poA[:], vs[pc][:, hA * 65:hA * 65 + 128],
                                ppt[:, 0:TQ], start=st, stop=False)
                            nc.tensor.matmul(
                                poB[:], vs[pc][:, hB * 65:hB * 65 + 128],
                                ppt[:, TQ:2 * TQ], start=st, stop=False)
                        pend = (c, pt)
                    pc, ppt = pend
                    one = (len(cs) == 1)
                    nc.tensor.matmul(poA[:], vs[pc][:, hA * 65:hA * 65 + 128],
                                     ppt[:, 0:TQ], start=one, stop=True)
                    nc.tensor.matmul(poB[:], vs[pc][:, hB * 65:hB * 65 + 128],
                                     ppt[:, TQ:2 * TQ], start=one, stop=True)

                    # normalize both heads: copy sums+O off PSUM fast, then
                    # approx-reciprocal from SBUF (keeps PSUM slots rotating)
                    for h, po in ((hA, poA), (hB, poB)):
                        hp = h % 2
                        sums = npool.tile([1, TQ], F32, tag="sums",
                                          name=f"sm{j}_{h}")
                        nc.vector.tensor_copy(sums[:], po[64:65, :])
                        o_sb = npool.tile([64, TQ], F32, tag="o_sb",
                                          name=f"ob{j}_{h}")
                        nc.vector.tensor_copy(o_sb[:], po[0:64, :])
                        recip = npool.tile([1, TQ], F32, tag="recip",
                                           name=f"rc{j}_{h}")
                        scr = npool.tile([1, TQ], F32, tag="scr",
                                         name=f"sc{j}_{h}")
                        nc.vector.reciprocal_approx_accurate(
                            out=recip[:], in_=sums[:], scratch=scr[:])
                        recip_r = npool.tile([1, TQ], F32R, tag="recip_r",
                                             name=f"rr{j}_{h}")
                        nc.vector.tensor_copy(recip_r[:], recip[:])
                        pb = ps_b.tile([64, TQ], F32, tag="pb",
                                       name=f"pb{j}_{h}")
                        nc.tensor.matmul(pb[:], ones_r[:, 0:64], recip_r[:],
                                         start=True, stop=True)
                        nc.vector.tensor_mul(
                            yT[i][hp * 64:(hp + 1) * 64, jsl], o_sb[:], pb[:])

        # ---------------- phase 3: output projection ----------------
        with ExitStack() as ph3:
            wstage3 = ph3.enter_context(tc.tile_pool(name="wstage3", bufs=2))
            wpool3 = ph3.enter_context(tc.tile_pool(name="wpool3", bufs=1))
            opool = ph3.enter_context(tc.tile_pool(name="opool", bufs=3))
            ps3 = ph3.enter_context(tc.tile_pool(name="ps3", bufs=4, space="PSUM"))

            wp_r = []
            for k in range(4):
                st = wstage3.tile([128, D], F32, tag="wps", name=f"wps{k}")
                nc.sync.dma_start(st[:], wp_d[k * 128:(k + 1) * 128, :])
                wr = wpool3.tile([128, D], F32R, tag=f"wp{k}", name=f"wp{k}")
                nc.vector.tensor_copy(wr[:], st[:])
                wp_r.append(wr)

            for t in range(T // 128):
                ot = opool.tile([128, D], F32, tag="ot", name=f"ot{t}")
                for nb in range(2):
                    ps = ps3.tile([128, 512], F32, tag="ps3",
                                  name=f"ps3_{t}_{nb}")
                    for k in range(4):
                        nc.tensor.matmul(
                            ps[:],
                            yT[k][:, t * 128:(t + 1) * 128],
                            wp_r[k][:, nb * 512:(nb + 1) * 512],
                            start=(k == 0), stop=(k == 3))
                    nc.scalar.copy(ot[:, nb * 512:(nb + 1) * 512], ps[:])
                nc.sync.dma_start(out_d[t * 128:(t + 1) * 128, :], ot[:])

    nc.compile()
    return nc


def _get_nc(causal: bool):
    if causal not in _CACHE:
        _CACHE[causal] = _build(causal)
    return _CACHE[causal]


def _host_masks() -> np.ndarray:
    i = np.arange(TKC)[:, None]
    jj = np.arange(TQ)[None, :]
    blocks = [(jj >= i + s * TKC).astype(np.float32) for s in range(4)]
    return np.ascontiguousarray(np.concatenate(blocks, axis=1))


def _make_in_maps(x, W_qkv, b_qkv, W_proj):
    masks_np = _host_masks()
    in_maps = []
    for core in range(N_CORES):
        b, g = core // 2, core % 2
        qc = slice(g * DL, (g + 1) * DL)
        kc = slice(D + g * DL, D + (g + 1) * DL)
        vc = slice(2 * D + g * DL, 2 * D + (g + 1) * DL)
        in_maps.append({
            "xT": np.ascontiguousarray(x[b].T),
            "wqk": np.ascontiguousarray(
                np.concatenate([W_qkv[:, qc], W_qkv[:, kc]], axis=1)),
            "wv": np.ascontiguousarray(W_qkv[:, vc]),
            "bqk": np.ascontiguousarray(
                np.concatenate([b_qkv[qc], b_qkv[kc]]).reshape(8, 128, 1)),
            "bv": np.ascontiguousarray(b_qkv[vc].reshape(1, DL)),
            "wproj": np.ascontiguousarray(W_proj[g * DL:(g + 1) * DL, :]),
            "masks": masks_np,
        })
    return in_maps


def kernel(x, mask, W_qkv, b_qkv, W_proj, b_proj):
    x = np.asarray(x, dtype=np.float32)
    mask2d = np.asarray(mask, dtype=np.int32).reshape(T, T)
    W_qkv = np.asarray(W_qkv, dtype=np.float32)
    b_qkv = np.asarray(b_qkv, dtype=np.float32)
    W_proj = np.asarray(W_proj, dtype=np.float32)
    b_proj = np.asarray(b_proj, dtype=np.float32)

    if np.array_equal(mask2d, np.tril(np.ones((T, T), dtype=np.int32))):
        causal = True
    elif np.all(mask2d == 1):
        causal = False
    else:
        raise NotImplementedError("only causal (tril) or all-ones masks")

    nc = _get_nc(causal)
    in_maps = _make_in_maps(x, W_qkv, b_qkv, W_proj)
    res = run_bass_kernel_spmd(nc, in_maps, core_ids=list(range(N_CORES)))
    out = np.empty((B, T, D), dtype=np.float32)
    for b in range(B):
        out[b] = (res.results[2 * b]["out"] + res.results[2 * b + 1]["out"]
                  + b_proj[None, :])
    return out
